# revision 4
# baseline (speedup 1.0000x reference)
"""ColumnParallelLinear kernel for Trainium2 (8 NeuronCores).

Computes Y[s,b,o] = sum_h X[s,b,h] * W[o,h]  (F.linear / einsum 'sbh,oh->sbo')
with S,B,H,OUT = 2048,4,1024,4096, fp32.

Strategy:
  - Flatten tokens: M = S*B = 8192 rows.  GEMM: [M,H] @ [H,OUT].
  - 2D shard over 8 cores: 4 token groups (2048 rows) x 2 out-column
    groups (2048 cols).  This minimizes per-core HBM traffic
    (x 8.4MB + w 8.4MB + y 16.8MB = 33.6MB/core) vs pure column- or
    row-parallel, keeping the kernel compute-bound.
  - Host pre-transposes X and W so the contraction dim (h) is
    outermost; on-chip tiles then have h on SBUF partitions with fully
    contiguous DMA, no on-device transposes.
  - Matmuls run as float32r (fp32 bits, full-rate PE path; moving dim
    512), accumulating fp32 in PSUM.
"""

import numpy as np

import concourse.bass as bass
from concourse import bacc
import concourse.mybir as mybir
import concourse.tile as tile
from concourse.bass_utils import run_bass_kernel_spmd

S, B, H, OUT = 2048, 4, 1024, 4096
M = S * B

N_CORES = 8
G_ROW, G_COL = 4, 2          # token groups x out-feature groups
M_LOC = M // G_ROW           # 2048 rows per core
N_LOC = OUT // G_COL         # 2048 out features per core

P = 128
KO = H // P                  # 8 contraction subtiles
MO = M_LOC // P              # 16 row tiles
NT = 512                     # psum free dim (one bank of fp32)
NO = N_LOC // NT             # 4 col tiles

MM_DT = mybir.dt.float32r    # full-rate fp32 matmul path


def build_nc(mm_dt=MM_DT):
    nc = bacc.Bacc(None, target_bir_lowering=False)
    xT = nc.declare_dram_parameter("xT", [H, M_LOC], mybir.dt.float32, isOutput=False)
    wT = nc.declare_dram_parameter("wT", [H, N_LOC], mybir.dt.float32, isOutput=False)
    y = nc.declare_dram_parameter("y", [M_LOC, N_LOC], mybir.dt.float32, isOutput=True)

    # h on partitions: row h = ko*P + p
    xT_r = xT[:, :].rearrange("(ko p) m -> p ko m", p=P)
    wT_r = wT[:, :].rearrange("(ko p) n -> p ko n", p=P)
    y_r = y[:, :].rearrange("(mo p) n -> p mo n", p=P)

    with tile.TileContext(nc) as tc:
        with (
            tc.tile_pool(name="xp", bufs=1) as xp,
            tc.tile_pool(name="wp", bufs=2) as wp,
            tc.tile_pool(name="op", bufs=4) as op,
            tc.tile_pool(name="psp", bufs=4, space="PSUM") as psp,
        ):
            # x: one tile per (mo-group, k-subtile), issued mo-group-major,
            # so the first row tiles complete quickly and the PE ramps while
            # the rest of x streams in.
            XG = 512                       # mo-group width (4 row tiles)
            NXG = M_LOC // XG
            x_sb = [[None] * KO for _ in range(NXG)]
            for g in range(NXG):
                for k in range(KO):
                    xk = xp.tile([P, XG], mm_dt, tag=f"x{g}_{k}")
                    nc.sync.dma_start(
                        xk[:], xT_r[:, k, g * XG:(g + 1) * XG].bitcast(mm_dt)
                    )
                    x_sb[g][k] = xk

            for n in range(NO):
                w_sb = []
                for k in range(KO):
                    wk = wp.tile([P, NT], mm_dt, tag=f"w{k}")
                    nc.sync.dma_start(wk[:], wT_r[:, k, n * NT:(n + 1) * NT].bitcast(mm_dt))
                    w_sb.append(wk)
                for mo in range(MO):
                    ps = psp.tile([P, NT], mybir.dt.float32)
                    for k in range(KO):
                        nc.tensor.matmul(
                            ps[:],
                            lhsT=x_sb[mo // 4][k][:, (mo % 4) * P:(mo % 4 + 1) * P],
                            rhs=w_sb[k][:],
                            start=(k == 0),
                            stop=(k == KO - 1),
                        )
                    o_sb = op.tile([P, NT], mybir.dt.float32)
                    nc.vector.tensor_copy(o_sb[:], ps[:])
                    nc.sync.dma_start(y_r[:, mo, n * NT:(n + 1) * NT], o_sb[:])
    nc.compile()
    return nc


def make_in_maps(input_, weight):
    X = np.ascontiguousarray(np.asarray(input_, dtype=np.float32).reshape(M, H))
    XT = np.ascontiguousarray(X.T)                                   # [H, M]
    WT = np.ascontiguousarray(np.asarray(weight, dtype=np.float32).T)  # [H, OUT]
    in_maps = []
    for c in range(N_CORES):
        i, j = divmod(c, G_COL)
        in_maps.append({
            "xT": np.ascontiguousarray(XT[:, i * M_LOC:(i + 1) * M_LOC]),
            "wT": np.ascontiguousarray(WT[:, j * N_LOC:(j + 1) * N_LOC]),
        })
    return in_maps


def assemble(results):
    Y = np.empty((M, OUT), dtype=np.float32)
    for c in range(N_CORES):
        i, j = divmod(c, G_COL)
        Y[i * M_LOC:(i + 1) * M_LOC, j * N_LOC:(j + 1) * N_LOC] = results[c]["y"]
    return Y.reshape(S, B, OUT)


def kernel(input_, weight):
    nc = build_nc()
    res = run_bass_kernel_spmd(nc, make_in_maps(input_, weight), list(range(N_CORES)))
    return assemble(res.results)
